# revision 25
# baseline (speedup 1.0000x reference)
"""AdaptiveLTCCell Trainium2 kernel (8 NeuronCores, batch-parallel).

Math per scan iteration (reference: 3 semi-implicit Euler unfolds + ACT halting):
    f  = sigmoid(x@Wx + h@Wh + b)
    h' = (h + dt*f*A) / (1 + dt*(1/tau + f))
Device formulation uses the scaled state S = h/dt with Ws = dt*Wh:
    z   = S@Ws + XP,         XP = x@Wx + b   (computed once on device)
    f   = sigmoid(z)
    S'  = (S + A*f) * rho,   rho = 1/(f + c''),  c'' = (1 + dt/tau)/dt
    h'  = dt*S'  (dt folded into halting/output coefficients)
Matmuls run as fp16 hi/lo splits accumulated in fp32 PSUM (error ~1e-6,
validated against exact fp64). The scan is truncated to N_STEPS iterations;
iterations past the point where every batch row has halted contribute exactly
zero to all outputs. The kernel emits the final `cont` mask and falls back to
a full-length build if any row is still continuing (never happens for inputs
where all rows halt within N_STEPS).
"""
import numpy as np
from contextlib import ExitStack

import concourse.bass as bass
import concourse.tile as tile
from concourse import bacc, mybir
from concourse.bass_utils import run_bass_kernel_spmd

f32 = mybir.dt.float32
f16 = mybir.dt.float16
AF = mybir.ActivationFunctionType
ALU = mybir.AluOpType
AX = mybir.AxisListType

B, D_IN, S = 1024, 1024, 1024
TIME_LIMIT = 16
ODE_UNFOLDS = 3
PONDER_EPS = 0.01
TIME_PENALTY = 0.01
N_CORES = 8
BL = B // N_CORES          # local batch rows per core
N_STEPS_FAST = 5           # all rows halt within this for the target input (cont-checked)
W_TERMS = 1                # 1: z=hi@W16; 2: z=hi@Whi+hi@Wlo; 3: +lo@Whi

DT64 = 1.0 / ODE_UNFOLDS
DT32 = np.float32(DT64)


def _build(n_steps: int, w_terms: int, first_zero: bool, tau_const: bool,
           fold_a: bool = False):
    nc = bacc.Bacc("TRN2", target_bir_lowering=False, debug=False,
                   num_devices=N_CORES)

    dram = {}
    def din(name, shape, dt=f32):
        dram[name] = nc.dram_tensor(name, list(shape), dt, kind="ExternalInput").ap()
        return dram[name]

    # fp16 weight splits (host-prepared), k-tile t occupies rows 128t:128(t+1)
    d_whi = din("whi", [S, S], f16)
    if w_terms >= 2:
        d_wlo = din("wlo", [S, S], f16)
    d_wxhi = din("wxhi", [D_IN, S], f16)
    d_wxlo = din("wxlo", [D_IN, S], f16)
    d_xhiT = din("xhiT", [D_IN, BL], f16)
    d_xloT = din("xloT", [D_IN, BL], f16)
    d_ident = din("ident", [128, 128], f16)      # fp16 identity (for XP replay MMs)
    d_identf = din("identf", [128, 128], f32)    # fp32 identity (for transposes)
    d_bvec = din("bvec", [1, S])                 # bias b as a row
    d_ones = din("ones", [1, 128])
    d_ab = din("ab", [128, S])                   # broadcast dt*A... actually A (state scaled)
    d_whs = din("whs", [128, S])                 # broadcast dt*w_halt
    d_bhalt = din("bhalt", [128, 1])
    d_consts = din("consts", [128, 4])           # [c'', budget, dt, 0]
    if not tau_const:
        d_cb = din("cb", [128, S])               # broadcast c'' (general tau)
    if not first_zero:
        d_s0 = din("s0", [BL, S])                # initial S = hidden0/dt
        d_s0hiT = din("s0hiT", [S, BL], f16)
        if w_terms >= 3:
            d_s0loT = din("s0loT", [S, BL], f16)

    o_toth = nc.dram_tensor("tot_h", [BL, S], f32, kind="ExternalOutput").ap()
    o_aux = nc.dram_tensor("aux", [BL, 4], f32, kind="ExternalOutput").ap()

    with tile.TileContext(nc) as tc, ExitStack() as ctx:
        cp = ctx.enter_context(tc.tile_pool(name="cp", bufs=1))
        sp = ctx.enter_context(tc.tile_pool(name="sp", bufs=4))
        tiny = ctx.enter_context(tc.tile_pool(name="tiny", bufs=2))
        pz = ctx.enter_context(tc.tile_pool(name="pz", bufs=4, space="PSUM"))
        pt = ctx.enter_context(tc.tile_pool(name="pt", bufs=4, space="PSUM"))

        # ---------------- constant loads ----------------
        def load_tiled(dst, dsrc, free, dt_):
            half = 4 * free
            src3 = dsrc.rearrange("(t p) f -> p t f", p=128)
            nc.sync.dma_start(dst[:, 0:half].rearrange("p (t f) -> p t f", t=4),
                              src3[:, 0:4, :])
            nc.sync.dma_start(dst[:, half:2 * half].rearrange("p (t f) -> p t f", t=4),
                              src3[:, 4:8, :])
        whi_sb = cp.tile([128, 8 * S], f16)
        load_tiled(whi_sb[:], d_whi, S, f16)
        if w_terms >= 2:
            wlo_sb = cp.tile([128, 8 * S], f16)
            load_tiled(wlo_sb[:], d_wlo, S, f16)
        wxhi_sb = cp.tile([128, 8 * S], f16)
        wxlo_sb = cp.tile([128, 8 * S], f16)
        load_tiled(wxhi_sb[:], d_wxhi, S, f16)
        load_tiled(wxlo_sb[:], d_wxlo, S, f16)
        xhiT_sb = cp.tile([128, 8 * BL], f16)
        load_tiled(xhiT_sb[:], d_xhiT, BL, f16)
        id16_sb = cp.tile([128, 128], f16)
        nc.sync.dma_start(id16_sb[:], d_ident[:])
        id32_sb = cp.tile([128, 128], f32)
        nc.sync.dma_start(id32_sb[:], d_identf[:])
        bvec_sb = cp.tile([1, S], f32)
        nc.sync.dma_start(bvec_sb[:], d_bvec[:])
        ones_sb = cp.tile([1, 128], f32)
        nc.sync.dma_start(ones_sb[:], d_ones[:])
        ab_sb = cp.tile([128, S], f32)
        nc.sync.dma_start(ab_sb[:], d_ab[:])
        whs_sb = cp.tile([128, S], f32)
        nc.sync.dma_start(whs_sb[:], d_whs[:])
        bhalt_sb = cp.tile([128, 1], f32)
        nc.sync.dma_start(bhalt_sb[:], d_bhalt[:])
        consts_sb = cp.tile([128, 4], f32)
        nc.sync.dma_start(consts_sb[:], d_consts[:])
        if not tau_const:
            cb_sb = cp.tile([128, S], f32)
            nc.sync.dma_start(cb_sb[:], d_cb[:])

        # ---------------- XP = x@Wx + b (once, fp16 3-term) ----------------
        xp_hi = cp.tile([128, S], f16)          # fp16 split of XP for replay
        xp_lo = cp.tile([128, S], f16)
        xp_f32 = cp.tile([128, S], f32)
        for bk in range(2):
            xpp = pz.tile([128, 512], f32, tag="z", name=f"xp_ps_{bk}")
            nmm = 0
            for part in range(2):
                win = wxhi_sb if part == 0 else wxlo_sb
                for k in range(8):
                    nmm += 1
                    nc.tensor.matmul(
                        xpp[:],
                        xhiT_sb[:, BL * k:BL * (k + 1)],
                        win[:, S * k + 512 * bk:S * k + 512 * bk + 512],
                        start=(nmm == 1), stop=(nmm == 16))
            # add bias b via K=1 outer product (rank-1), separate group
            nc.tensor.matmul(xpp[:], ones_sb[0:1, 0:128].bitcast(f32),
                             bvec_sb[0:1, 512 * bk:512 * bk + 512],
                             start=False, stop=True, skip_group_check=True)
            sl = slice(512 * bk, 512 * (bk + 1))
            nc.vector.tensor_copy(xp_f32[:, sl], xpp[:])
            nc.vector.tensor_copy(xp_hi[:, sl], xpp[:])
        nc.vector.tensor_sub(xp_lo[:], xp_f32[:], xp_hi[:])

        # ---------------- state init ----------------
        s_bufs = [cp.tile([128, S], f32, name="s_a"), cp.tile([128, S], f32, name="s_b")]
        sht_bufs = [cp.tile([128, S], f16, name="sht_a"), cp.tile([128, S], f16, name="sht_b")]
        if w_terms >= 3:
            slt_bufs = [cp.tile([128, S], f16, name="slt_a"), cp.tile([128, S], f16, name="slt_b")]
        toth_bufs = [cp.tile([128, S], f32, name="toth_a"), cp.tile([128, S], f32, name="toth_b")]
        if first_zero:
            nc.vector.memset(s_bufs[0][:], 0.0)
        else:
            nc.sync.dma_start(s_bufs[0][:], d_s0[:])
            load_tiled(sht_bufs[0][:], d_s0hiT, BL, f16)
            if w_terms >= 3:
                load_tiled(slt_bufs[0][:], d_s0loT, BL, f16)
        nc.vector.memset(toth_bufs[0][:], 0.0)

        cont = cp.tile([128, 1], f32, name="cont0")
        nc.vector.memset(cont[:], 1.0)
        halt_acc = cp.tile([128, 1], f32, name="ha0")
        nc.vector.memset(halt_acc[:], 0.0)
        tot_steps = cp.tile([128, 1], f32, name="ts0")
        nc.vector.memset(tot_steps[:], 0.0)
        tot_rem = cp.tile([128, 1], f32, name="tr0")
        nc.vector.memset(tot_rem[:], 0.0)

        cpp = consts_sb  # [c'', budget, dt, 0]
        s_cur = 0
        toth_cur = 0

        for step in range(n_steps):
            for u in range(ODE_UNFOLDS):
                uid = step * ODE_UNFOLDS + u
                skip_smm = first_zero and uid == 0
                f_sb = sp.tile([128, S], f32, name=f"f_{uid}", tag="f")
                den_sb = sp.tile([128, S], f32, name=f"den_{uid}", tag="den")
                rho_sb = sp.tile([128, S], f32, name=f"rho_{uid}", tag="rho")
                fa_sb = sp.tile([128, S], f32, name=f"fa_{uid}", tag="fa")
                num_sb = sp.tile([128, S], f32, name=f"num_{uid}", tag="num")
                s_new = s_bufs[1 - s_cur]
                sht_old = sht_bufs[s_cur]
                sht_new = sht_bufs[1 - s_cur]
                if w_terms >= 3:
                    slt_old = slt_bufs[s_cur]
                    slt_new = slt_bufs[1 - s_cur]
                s_old = s_bufs[s_cur]

                for bk in range(2):
                    zp = pz.tile([128, 512], f32, tag="z", name=f"z_{uid}_{bk}")
                    # XP replay (identity MMs, fp16 hi+lo)
                    nc.tensor.matmul(zp[:], id16_sb[:], xp_hi[:, 512 * bk:512 * bk + 512],
                                     start=True, stop=False)
                    last = skip_smm
                    nc.tensor.matmul(zp[:], id16_sb[:], xp_lo[:, 512 * bk:512 * bk + 512],
                                     start=False, stop=last)
                    if not skip_smm:
                        nmm = 0
                        tot_mm = 8 * w_terms
                        for k in range(8):
                            ksl = slice(128 * k, 128 * (k + 1))
                            wsl = slice(S * k + 512 * bk, S * k + 512 * bk + 512)
                            nmm += 1
                            nc.tensor.matmul(zp[:], sht_old[:, ksl], whi_sb[:, wsl],
                                             start=False, stop=(nmm == tot_mm))
                            if w_terms >= 2:
                                nmm += 1
                                nc.tensor.matmul(zp[:], sht_old[:, ksl], wlo_sb[:, wsl],
                                                 start=False, stop=(nmm == tot_mm))
                            if w_terms >= 3:
                                nmm += 1
                                nc.tensor.matmul(zp[:], slt_old[:, ksl], whi_sb[:, wsl],
                                                 start=False, stop=(nmm == tot_mm))
                    last_u = (step == n_steps - 1 and u == ODE_UNFOLDS - 1)
                    sl = slice(512 * bk, 512 * (bk + 1))
                    stp = None
                    if not last_u:
                        stp = pt.tile([128, 512], f32, tag="st", name=f"st_{uid}_{bk}")
                    for q in range(2):
                        qsl = slice(512 * bk + 256 * q, 512 * bk + 256 * (q + 1))
                        zq = slice(256 * q, 256 * (q + 1))
                        # f = sigmoid(z)
                        nc.scalar.activation(f_sb[:, qsl], zp[:, zq], AF.Sigmoid)
                        # den = dt*f + (1 + dt/tau)
                        if tau_const:
                            nc.scalar.activation(den_sb[:, qsl], f_sb[:, qsl], AF.Identity,
                                                 scale=cpp[:, 2:3], bias=cpp[:, 0:1])
                        else:
                            nc.vector.scalar_tensor_tensor(
                                out=den_sb[:, qsl], in0=f_sb[:, qsl], scalar=float(DT32),
                                in1=cb_sb[:, qsl], op0=ALU.mult, op1=ALU.add)
                        # rho ~= 1/den (51-ULP fast reciprocal)
                        nc.vector.reciprocal_approx_fast(out=rho_sb[:, qsl], in_=den_sb[:, qsl])
                        if fold_a:
                            # T-space: T' = (T + f) * rho  (A folded into weights)
                            if skip_smm and first_zero:
                                nc.vector.tensor_mul(s_new[:, qsl], f_sb[:, qsl], rho_sb[:, qsl])
                            else:
                                nc.vector.tensor_add(num_sb[:, qsl], f_sb[:, qsl], s_old[:, qsl])
                                nc.vector.tensor_mul(s_new[:, qsl], num_sb[:, qsl], rho_sb[:, qsl])
                        else:
                            # fa = f * A ; num = S + fa ; S' = num * rho
                            nc.vector.tensor_mul(fa_sb[:, qsl], f_sb[:, qsl], ab_sb[:, qsl])
                            if skip_smm and first_zero:
                                nc.vector.tensor_mul(s_new[:, qsl], fa_sb[:, qsl], rho_sb[:, qsl])
                            else:
                                nc.vector.tensor_add(num_sb[:, qsl], fa_sb[:, qsl], s_old[:, qsl])
                                nc.vector.tensor_mul(s_new[:, qsl], num_sb[:, qsl], rho_sb[:, qsl])
                        # transpose S' tiles (fp32) and cast to fp16 for next unfold
                        if not last_u:
                            for t2 in range(2):
                                t4 = 2 * q + t2
                                t = 4 * bk + t4
                                nc.tensor.matmul(
                                    stp[:, 128 * t4:128 * (t4 + 1)],
                                    s_new[:, 128 * t:128 * (t + 1)], id32_sb[:],
                                    is_transpose=True, start=True, stop=True)
                            nc.scalar.activation(
                                sht_new[:, qsl], stp[:, 256 * q:256 * (q + 1)], AF.Copy)
                            if w_terms >= 3:
                                nc.vector.tensor_sub(slt_new[:, qsl],
                                                     stp[:, 256 * q:256 * (q + 1)],
                                                     sht_new[:, qsl])
                s_cur = 1 - s_cur

            # ---------- halting (end of step) ----------
            s_now = s_bufs[s_cur]
            hv = tiny.tile([128, 1], f32, name=f"hv_{step}")
            ttr_junk = sp.tile([128, S], f32, name=f"ttrj_{step}", tag="ttrj")
            nc.vector.affine_mul_reduce(out=ttr_junk[:], accum_out=hv[:],
                                        in0=s_now[:], in1=whs_sb[:],
                                        scale=1.0, bias=0.0)
            sh = tiny.tile([128, 1], f32, name=f"sh_{step}")
            nc.scalar.activation(sh[:], hv[:], AF.Sigmoid, bias=bhalt_sb[:, 0:1])
            mh = tiny.tile([128, 1], f32, name=f"mh_{step}")
            nc.vector.tensor_mul(mh[:], cont[:], sh[:])
            ha_new = tiny.tile([128, 1], f32, name=f"ha_{step}")
            nc.vector.tensor_add(ha_new[:], halt_acc[:], mh[:])
            pre = tiny.tile([128, 1], f32, name=f"pre_{step}")
            nc.vector.tensor_add(pre[:], ha_new[:], sh[:])
            endr = tiny.tile([128, 1], f32, name=f"endr_{step}")
            nc.vector.tensor_scalar(out=endr[:], in0=pre[:],
                                    scalar1=float(np.float32(1.0 - PONDER_EPS)),
                                    scalar2=None, op0=ALU.is_gt)
            ending = tiny.tile([128, 1], f32, name=f"end_{step}")
            nc.vector.tensor_mul(ending[:], cont[:], endr[:])
            cont_new = tiny.tile([128, 1], f32, name=f"cont_{step}")
            nc.vector.tensor_sub(cont_new[:], cont[:], ending[:])
            ts_new = tiny.tile([128, 1], f32, name=f"tsn_{step}")
            nc.vector.tensor_add(ts_new[:], tot_steps[:], cont_new[:])
            onemha = tiny.tile([128, 1], f32, name=f"omh_{step}")
            nc.vector.tensor_scalar(out=onemha[:], in0=ha_new[:], scalar1=-1.0,
                                    scalar2=1.0, op0=ALU.mult, op1=ALU.add)
            mr = tiny.tile([128, 1], f32, name=f"mr_{step}")
            nc.vector.tensor_mul(mr[:], ending[:], onemha[:])
            wsum = tiny.tile([128, 1], f32, name=f"w_{step}")
            nc.vector.tensor_add(wsum[:], mh[:], mr[:])
            wdt = tiny.tile([128, 1], f32, name=f"wdt_{step}")
            nc.vector.tensor_scalar(out=wdt[:], in0=wsum[:], scalar1=float(DT32),
                                    scalar2=None, op0=ALU.mult)
            tr_new = tiny.tile([128, 1], f32, name=f"trn_{step}")
            nc.vector.tensor_add(tr_new[:], tot_rem[:], mh[:])
            # tot_h += wdt * S   (h = dt*S)
            th_new = toth_bufs[1 - toth_cur]
            nc.vector.scalar_tensor_tensor(
                out=th_new[:], in0=s_now[:], scalar=wdt[:, 0:1],
                in1=toth_bufs[toth_cur][:], op0=ALU.mult, op1=ALU.add)
            toth_cur = 1 - toth_cur
            cont, halt_acc, tot_steps, tot_rem = cont_new, ha_new, ts_new, tr_new

        # ---------- final remainder term ----------
        onemha_f = tiny.tile([128, 1], f32, name="omh_f")
        nc.vector.tensor_scalar(out=onemha_f[:], in0=halt_acc[:], scalar1=-1.0,
                                scalar2=1.0, op0=ALU.mult, op1=ALU.add)
        wf = tiny.tile([128, 1], f32, name="wf")
        nc.vector.tensor_mul(wf[:], cont[:], onemha_f[:])
        wfdt = tiny.tile([128, 1], f32, name="wfdt")
        nc.vector.tensor_scalar(out=wfdt[:], in0=wf[:], scalar1=float(DT32),
                                scalar2=None, op0=ALU.mult)
        th_fin = toth_bufs[1 - toth_cur]
        nc.vector.scalar_tensor_tensor(
            out=th_fin[:], in0=s_bufs[s_cur][:], scalar=wfdt[:, 0:1],
            in1=toth_bufs[toth_cur][:], op0=ALU.mult, op1=ALU.add)

        if fold_a:
            th_conv = cp.tile([128, S], f32, name="th_conv")
            nc.vector.tensor_mul(th_conv[:], th_fin[:], ab_sb[:])
            th_fin = th_conv

        aux_sb = cp.tile([128, 4], f32, name="aux_sb")
        nc.vector.tensor_copy(aux_sb[:, 0:1], tot_rem[:])
        nc.vector.tensor_copy(aux_sb[:, 1:2], tot_steps[:])
        nc.vector.tensor_copy(aux_sb[:, 2:3], cont[:])
        nc.vector.tensor_copy(aux_sb[:, 3:4], halt_acc[:])

        nc.sync.dma_start(o_toth[:], th_fin[:])
        nc.sync.dma_start(o_aux[:], aux_sb[:])

    nc.compile()
    return nc


_BUILD_CACHE = {}


def _get_program(n_steps, w_terms, first_zero, tau_const, fold_a):
    key = (n_steps, w_terms, first_zero, tau_const, fold_a)
    if key not in _BUILD_CACHE:
        _BUILD_CACHE[key] = _build(n_steps, w_terms, first_zero, tau_const,
                                   fold_a=fold_a)
    return _BUILD_CACHE[key]


def _prep_inputs(inputs, hidden0, Wx, Wh, b, A, tau, w_halt, b_halt,
                 w_terms, first_zero, tau_const, fold_a):
    ws = (DT32 * Wh.astype(np.float64)).astype(np.float32)
    if fold_a:
        # T-space: fold A into weight rows / halting weights; state T = S/A
        ws = (A.astype(np.float64).reshape(-1, 1) * ws).astype(np.float32)
    whi = ws.astype(np.float16)
    wlo = (ws - whi.astype(np.float32)).astype(np.float16)
    wxhi = Wx.astype(np.float16)
    wxlo = (Wx - wxhi.astype(np.float32)).astype(np.float16)

    cpp = np.float32(1.0 + DT64 / tau.astype(np.float64))
    abv = A.astype(np.float32)          # S' = (S + A*f)*rho  (state scaled by 1/dt)
    whs64 = DT32 * w_halt.astype(np.float64).reshape(-1)
    if fold_a:
        whs64 = whs64 * A.astype(np.float64)
    whs = whs64.astype(np.float32).reshape(1, S)

    common = {
        "whi": whi, "wxhi": wxhi, "wxlo": wxlo,
        "ident": np.eye(128, dtype=np.float16),
        "identf": np.eye(128, dtype=np.float32),
        "bvec": b.reshape(1, S).astype(np.float32),
        "ones": np.ones((1, 128), np.float32),
        "ab": np.broadcast_to(abv.reshape(1, S), (128, S)).copy(),
        "whs": np.broadcast_to(whs, (128, S)).copy(),
        "bhalt": np.full((128, 1), np.float32(b_halt.reshape(-1)[0]), np.float32),
        "consts": np.broadcast_to(
            np.array([[cpp.flat[0] if tau_const else 0.0,
                       np.float32(1.0 - PONDER_EPS), DT32, 0.0]], np.float32),
            (128, 4)).copy(),
    }
    if w_terms >= 2:
        common["wlo"] = wlo
    if not tau_const:
        common["cb"] = np.broadcast_to(cpp.reshape(1, S), (128, S)).copy()

    in_maps = []
    for c in range(N_CORES):
        rows = slice(BL * c, BL * (c + 1))
        xs = inputs[rows, :]
        xT = np.ascontiguousarray(xs.T)
        xhiT = xT.astype(np.float16)
        xloT = (xT - xhiT.astype(np.float32)).astype(np.float16)
        m = dict(common)
        m["xhiT"] = xhiT
        m["xloT"] = xloT
        if not first_zero:
            s0 = (hidden0[rows, :].astype(np.float64) / DT64)
            if fold_a:
                s0 = s0 / A.astype(np.float64).reshape(1, -1)
            s0 = s0.astype(np.float32)
            s0hiT = np.ascontiguousarray(s0.T).astype(np.float16)
            m["s0"] = s0
            m["s0hiT"] = s0hiT
            if w_terms >= 3:
                m["s0loT"] = (np.ascontiguousarray(s0.T)
                              - s0hiT.astype(np.float32)).astype(np.float16)
        in_maps.append(m)
    return in_maps


def _run(n_steps, w_terms, first_zero, tau_const, in_maps, trace=False, fold_a=False):
    nc = _get_program(n_steps, w_terms, first_zero, tau_const, fold_a)
    res = run_bass_kernel_spmd(nc, in_maps, list(range(N_CORES)), trace=trace,
                               trace_cores=[0] if trace else None)
    return res


def kernel(inputs, hidden0, Wx, Wh, b, A, tau, w_halt, b_halt,
           _trace=False, _n_steps=N_STEPS_FAST, _w_terms=W_TERMS):
    inputs = np.asarray(inputs, np.float32)
    hidden0 = np.asarray(hidden0, np.float32)
    Wx = np.asarray(Wx, np.float32)
    Wh = np.asarray(Wh, np.float32)
    b = np.asarray(b, np.float32)
    A = np.asarray(A, np.float32)
    tau = np.asarray(tau, np.float32)
    w_halt = np.asarray(w_halt, np.float32)
    b_halt = np.asarray(b_halt, np.float32)

    first_zero = not hidden0.any()
    tau_const = bool(np.all(tau == tau.flat[0]))
    fold_a = bool(np.abs(A).min() >= 2e-4)   # |T|=|S/A| must stay well inside fp16

    in_maps = _prep_inputs(inputs, hidden0, Wx, Wh, b, A, tau, w_halt, b_halt,
                           _w_terms, first_zero, tau_const, fold_a)

    res = _run(_n_steps, _w_terms, first_zero, tau_const, in_maps, trace=_trace,
               fold_a=fold_a)
    results = res.results

    toth = np.concatenate([results[c]["tot_h"] for c in range(N_CORES)], axis=0)
    aux = np.concatenate([results[c]["aux"] for c in range(N_CORES)], axis=0)
    tot_rem, tot_steps, cont = aux[:, 0:1], aux[:, 1:2], aux[:, 2:3]

    if cont.any() and _n_steps < TIME_LIMIT - 1:
        # some rows did not halt inside the truncated scan: rerun full length
        res = _run(TIME_LIMIT - 1, _w_terms, first_zero, tau_const, in_maps,
                   trace=_trace, fold_a=fold_a)
        results = res.results
        toth = np.concatenate([results[c]["tot_h"] for c in range(N_CORES)], axis=0)
        aux = np.concatenate([results[c]["aux"] for c in range(N_CORES)], axis=0)
        tot_rem, tot_steps, cont = aux[:, 0:1], aux[:, 1:2], aux[:, 2:3]

    ponder_cost = np.float32(-TIME_PENALTY * tot_rem.astype(np.float64).mean())
    out_steps = (tot_steps + 1.0).astype(np.float32)
    kernel._last_exec_time_ns = getattr(res, "exec_time_ns", None)
    kernel._last_res = res
    return toth.astype(np.float32), ponder_cost, out_steps


# revision 26
# speedup vs baseline: 1.0589x; 1.0589x over previous
"""AdaptiveLTCCell Trainium2 kernel (8 NeuronCores, batch-parallel).

Math per scan iteration (reference: 3 semi-implicit Euler unfolds + ACT halting):
    f  = sigmoid(x@Wx + h@Wh + b)
    h' = (h + dt*f*A) / (1 + dt*(1/tau + f))
Device formulation uses the scaled state S = h/dt with Ws = dt*Wh:
    z   = S@Ws + XP,         XP = x@Wx + b   (computed once on device)
    f   = sigmoid(z)
    S'  = (S + A*f) * rho,   rho = 1/(f + c''),  c'' = (1 + dt/tau)/dt
    h'  = dt*S'  (dt folded into halting/output coefficients)
Matmuls run as fp16 hi/lo splits accumulated in fp32 PSUM (error ~1e-6,
validated against exact fp64). The scan is truncated to N_STEPS iterations;
iterations past the point where every batch row has halted contribute exactly
zero to all outputs. The kernel emits the final `cont` mask and falls back to
a full-length build if any row is still continuing (never happens for inputs
where all rows halt within N_STEPS).
"""
import numpy as np
from contextlib import ExitStack

import concourse.bass as bass
import concourse.tile as tile
from concourse import bacc, mybir
from concourse.bass_utils import run_bass_kernel_spmd

f32 = mybir.dt.float32
f16 = mybir.dt.float16
AF = mybir.ActivationFunctionType
ALU = mybir.AluOpType
AX = mybir.AxisListType

B, D_IN, S = 1024, 1024, 1024
TIME_LIMIT = 16
ODE_UNFOLDS = 3
PONDER_EPS = 0.01
TIME_PENALTY = 0.01
N_CORES = 8
BL = B // N_CORES          # local batch rows per core
N_STEPS_FAST = 5           # all rows halt within this for the target input (cont-checked)
W_TERMS = 1                # 1: z=hi@W16; 2: z=hi@Whi+hi@Wlo; 3: +lo@Whi

DT64 = 1.0 / ODE_UNFOLDS
DT32 = np.float32(DT64)


def _build(n_steps: int, w_terms: int, first_zero: bool, tau_const: bool,
           fold_a: bool = False):
    nc = bacc.Bacc("TRN2", target_bir_lowering=False, debug=False,
                   num_devices=N_CORES)

    dram = {}
    def din(name, shape, dt=f32):
        dram[name] = nc.dram_tensor(name, list(shape), dt, kind="ExternalInput").ap()
        return dram[name]

    # fp16 weight splits (host-prepared), k-tile t occupies rows 128t:128(t+1)
    d_whi = din("whi", [S, S], f16)
    if w_terms >= 2:
        d_wlo = din("wlo", [S, S], f16)
    d_wxhi = din("wxhi", [D_IN, S], f16)
    d_wxlo = din("wxlo", [D_IN, S], f16)
    d_xhiT = din("xhiT", [D_IN, BL], f16)
    d_xloT = din("xloT", [D_IN, BL], f16)
    d_ident = din("ident", [128, 128], f16)      # fp16 identity (for XP replay MMs)
    d_identf = din("identf", [128, 128], f32)    # fp32 identity (for transposes)
    d_bvec = din("bvec", [1, S])                 # bias b as a row
    d_ones = din("ones", [1, 128])
    d_ab = din("ab", [128, S])                   # broadcast dt*A... actually A (state scaled)
    d_whs = din("whs", [128, S])                 # broadcast dt*w_halt
    d_bhalt = din("bhalt", [128, 1])
    d_consts = din("consts", [128, 4])           # [c'', budget, dt, 0]
    if not tau_const:
        d_cb = din("cb", [128, S])               # broadcast c'' (general tau)
    if not first_zero:
        d_s0 = din("s0", [BL, S])                # initial S = hidden0/dt
        d_s0hiT = din("s0hiT", [S, BL], f16)
        if w_terms >= 3:
            d_s0loT = din("s0loT", [S, BL], f16)

    o_toth = nc.dram_tensor("tot_h", [BL, S], f32, kind="ExternalOutput").ap()
    o_aux = nc.dram_tensor("aux", [BL, 4], f32, kind="ExternalOutput").ap()

    with tile.TileContext(nc) as tc, ExitStack() as ctx:
        cp = ctx.enter_context(tc.tile_pool(name="cp", bufs=1))
        sp = ctx.enter_context(tc.tile_pool(name="sp", bufs=3))
        tiny = ctx.enter_context(tc.tile_pool(name="tiny", bufs=2))
        pz = ctx.enter_context(tc.tile_pool(name="pz", bufs=4, space="PSUM"))
        pt = ctx.enter_context(tc.tile_pool(name="pt", bufs=4, space="PSUM"))

        # ---------------- constant loads ----------------
        def load_tiled(dst, dsrc, free, dt_):
            half = 4 * free
            src3 = dsrc.rearrange("(t p) f -> p t f", p=128)
            nc.sync.dma_start(dst[:, 0:half].rearrange("p (t f) -> p t f", t=4),
                              src3[:, 0:4, :])
            nc.sync.dma_start(dst[:, half:2 * half].rearrange("p (t f) -> p t f", t=4),
                              src3[:, 4:8, :])
        whi_sb = cp.tile([128, 8 * S], f16)
        load_tiled(whi_sb[:], d_whi, S, f16)
        if w_terms >= 2:
            wlo_sb = cp.tile([128, 8 * S], f16)
            load_tiled(wlo_sb[:], d_wlo, S, f16)
        wxhi_sb = cp.tile([128, 8 * S], f16)
        load_tiled(wxhi_sb[:], d_wxhi, S, f16)
        xhiT_sb = cp.tile([128, 8 * BL], f16)
        load_tiled(xhiT_sb[:], d_xhiT, BL, f16)
        id16_sb = cp.tile([128, 128], f16)
        nc.sync.dma_start(id16_sb[:], d_ident[:])
        id32_sb = cp.tile([128, 128], f32)
        nc.sync.dma_start(id32_sb[:], d_identf[:])
        bvec_sb = cp.tile([1, S], f32)
        nc.sync.dma_start(bvec_sb[:], d_bvec[:])
        ones_sb = cp.tile([1, 128], f32)
        nc.sync.dma_start(ones_sb[:], d_ones[:])
        ab_sb = cp.tile([128, S], f32)
        nc.sync.dma_start(ab_sb[:], d_ab[:])
        whs_sb = cp.tile([128, S], f32)
        nc.sync.dma_start(whs_sb[:], d_whs[:])
        bhalt_sb = cp.tile([128, 1], f32)
        nc.sync.dma_start(bhalt_sb[:], d_bhalt[:])
        consts_sb = cp.tile([128, 4], f32)
        nc.sync.dma_start(consts_sb[:], d_consts[:])
        if not tau_const:
            cb_sb = cp.tile([128, S], f32)
            nc.sync.dma_start(cb_sb[:], d_cb[:])

        # ---------------- XP = x@Wx + b (once, fp16 3-term) ----------------
        xp_hi = cp.tile([128, S], f16)          # fp16 split of XP for replay
        xp_lo = cp.tile([128, S], f16)
        xp_f32 = cp.tile([128, S], f32)
        for bk in range(2):
            xpp = pz.tile([128, 512], f32, tag="z", name=f"xp_ps_{bk}")
            for k in range(8):
                nc.tensor.matmul(
                    xpp[:],
                    xhiT_sb[:, BL * k:BL * (k + 1)],
                    wxhi_sb[:, S * k + 512 * bk:S * k + 512 * bk + 512],
                    start=(k == 0), stop=(k == 7))
            # add bias b via K=1 outer product (rank-1), separate group
            nc.tensor.matmul(xpp[:], ones_sb[0:1, 0:128].bitcast(f32),
                             bvec_sb[0:1, 512 * bk:512 * bk + 512],
                             start=False, stop=True, skip_group_check=True)
            sl = slice(512 * bk, 512 * (bk + 1))
            nc.vector.tensor_copy(xp_f32[:, sl], xpp[:])
            nc.vector.tensor_copy(xp_hi[:, sl], xpp[:])
        nc.vector.tensor_sub(xp_lo[:], xp_f32[:], xp_hi[:])

        # ---------------- state init ----------------
        s_bufs = [cp.tile([128, S], f32, name="s_a"), cp.tile([128, S], f32, name="s_b")]
        sht_bufs = [cp.tile([128, S], f16, name="sht_a"), cp.tile([128, S], f16, name="sht_b")]
        if w_terms >= 3:
            slt_bufs = [cp.tile([128, S], f16, name="slt_a"), cp.tile([128, S], f16, name="slt_b")]
        toth_bufs = [cp.tile([128, S], f32, name="toth_a"), cp.tile([128, S], f32, name="toth_b")]
        if first_zero:
            nc.vector.memset(s_bufs[0][:], 0.0)
        else:
            nc.sync.dma_start(s_bufs[0][:], d_s0[:])
            load_tiled(sht_bufs[0][:], d_s0hiT, BL, f16)
            if w_terms >= 3:
                load_tiled(slt_bufs[0][:], d_s0loT, BL, f16)
        nc.vector.memset(toth_bufs[0][:], 0.0)

        cont = cp.tile([128, 1], f32, name="cont0")
        nc.vector.memset(cont[:], 1.0)
        halt_acc = cp.tile([128, 1], f32, name="ha0")
        nc.vector.memset(halt_acc[:], 0.0)
        tot_steps = cp.tile([128, 1], f32, name="ts0")
        nc.vector.memset(tot_steps[:], 0.0)
        tot_rem = cp.tile([128, 1], f32, name="tr0")
        nc.vector.memset(tot_rem[:], 0.0)

        cpp = consts_sb  # [c'', budget, dt, 0]
        s_cur = 0
        toth_cur = 0

        for step in range(n_steps):
            for u in range(ODE_UNFOLDS):
                uid = step * ODE_UNFOLDS + u
                skip_smm = first_zero and uid == 0
                f_sb = sp.tile([128, S], f32, name=f"f_{uid}", tag="f")
                den_sb = sp.tile([128, S], f32, name=f"den_{uid}", tag="den")
                rho_sb = sp.tile([128, S], f32, name=f"rho_{uid}", tag="rho")
                fa_sb = sp.tile([128, S], f32, name=f"fa_{uid}", tag="fa")
                num_sb = sp.tile([128, S], f32, name=f"num_{uid}", tag="num")
                s_new = s_bufs[1 - s_cur]
                sht_old = sht_bufs[s_cur]
                sht_new = sht_bufs[1 - s_cur]
                if w_terms >= 3:
                    slt_old = slt_bufs[s_cur]
                    slt_new = slt_bufs[1 - s_cur]
                s_old = s_bufs[s_cur]

                for bk in range(2):
                    zp = pz.tile([128, 512], f32, tag="z", name=f"z_{uid}_{bk}")
                    # XP replay (identity MMs, fp16 hi+lo)
                    nc.tensor.matmul(zp[:], id16_sb[:], xp_hi[:, 512 * bk:512 * bk + 512],
                                     start=True, stop=False)
                    last = skip_smm
                    nc.tensor.matmul(zp[:], id16_sb[:], xp_lo[:, 512 * bk:512 * bk + 512],
                                     start=False, stop=last)
                    if not skip_smm:
                        nmm = 0
                        tot_mm = 8 * w_terms
                        for k in range(8):
                            ksl = slice(128 * k, 128 * (k + 1))
                            wsl = slice(S * k + 512 * bk, S * k + 512 * bk + 512)
                            nmm += 1
                            nc.tensor.matmul(zp[:], sht_old[:, ksl], whi_sb[:, wsl],
                                             start=False, stop=(nmm == tot_mm))
                            if w_terms >= 2:
                                nmm += 1
                                nc.tensor.matmul(zp[:], sht_old[:, ksl], wlo_sb[:, wsl],
                                                 start=False, stop=(nmm == tot_mm))
                            if w_terms >= 3:
                                nmm += 1
                                nc.tensor.matmul(zp[:], slt_old[:, ksl], whi_sb[:, wsl],
                                                 start=False, stop=(nmm == tot_mm))
                    last_u = (step == n_steps - 1 and u == ODE_UNFOLDS - 1)
                    sl = slice(512 * bk, 512 * (bk + 1))
                    stp = None
                    if not last_u:
                        stp = pt.tile([128, 512], f32, tag="st", name=f"st_{uid}_{bk}")
                    for q in range(2):
                        qsl = slice(512 * bk + 256 * q, 512 * bk + 256 * (q + 1))
                        zq = slice(256 * q, 256 * (q + 1))
                        # f = sigmoid(z)
                        nc.scalar.activation(f_sb[:, qsl], zp[:, zq], AF.Sigmoid)
                        # den = dt*f + (1 + dt/tau)
                        if tau_const:
                            nc.scalar.activation(den_sb[:, qsl], f_sb[:, qsl], AF.Identity,
                                                 scale=cpp[:, 2:3], bias=cpp[:, 0:1])
                        else:
                            nc.vector.scalar_tensor_tensor(
                                out=den_sb[:, qsl], in0=f_sb[:, qsl], scalar=float(DT32),
                                in1=cb_sb[:, qsl], op0=ALU.mult, op1=ALU.add)
                        # rho ~= 1/den (51-ULP fast reciprocal)
                        nc.vector.reciprocal_approx_fast(out=rho_sb[:, qsl], in_=den_sb[:, qsl])
                        if fold_a:
                            # T-space: T' = (T + f) * rho  (A folded into weights)
                            if skip_smm and first_zero:
                                nc.vector.tensor_mul(s_new[:, qsl], f_sb[:, qsl], rho_sb[:, qsl])
                            else:
                                nc.vector.tensor_add(num_sb[:, qsl], f_sb[:, qsl], s_old[:, qsl])
                                nc.vector.tensor_mul(s_new[:, qsl], num_sb[:, qsl], rho_sb[:, qsl])
                        else:
                            # fa = f * A ; num = S + fa ; S' = num * rho
                            nc.vector.tensor_mul(fa_sb[:, qsl], f_sb[:, qsl], ab_sb[:, qsl])
                            if skip_smm and first_zero:
                                nc.vector.tensor_mul(s_new[:, qsl], fa_sb[:, qsl], rho_sb[:, qsl])
                            else:
                                nc.vector.tensor_add(num_sb[:, qsl], fa_sb[:, qsl], s_old[:, qsl])
                                nc.vector.tensor_mul(s_new[:, qsl], num_sb[:, qsl], rho_sb[:, qsl])
                        # transpose S' tiles (fp32) and cast to fp16 for next unfold
                        if not last_u:
                            for t2 in range(2):
                                t4 = 2 * q + t2
                                t = 4 * bk + t4
                                nc.tensor.matmul(
                                    stp[:, 128 * t4:128 * (t4 + 1)],
                                    s_new[:, 128 * t:128 * (t + 1)], id32_sb[:],
                                    is_transpose=True, start=True, stop=True)
                            nc.scalar.activation(
                                sht_new[:, qsl], stp[:, 256 * q:256 * (q + 1)], AF.Copy)
                            if w_terms >= 3:
                                nc.vector.tensor_sub(slt_new[:, qsl],
                                                     stp[:, 256 * q:256 * (q + 1)],
                                                     sht_new[:, qsl])
                s_cur = 1 - s_cur

            # ---------- halting (end of step) ----------
            s_now = s_bufs[s_cur]
            hv = tiny.tile([128, 1], f32, name=f"hv_{step}")
            ttr_junk = sp.tile([128, S], f32, name=f"ttrj_{step}", tag="ttrj")
            nc.vector.affine_mul_reduce(out=ttr_junk[:], accum_out=hv[:],
                                        in0=s_now[:], in1=whs_sb[:],
                                        scale=1.0, bias=0.0)
            sh = tiny.tile([128, 1], f32, name=f"sh_{step}")
            nc.scalar.activation(sh[:], hv[:], AF.Sigmoid, bias=bhalt_sb[:, 0:1])
            mh = tiny.tile([128, 1], f32, name=f"mh_{step}")
            nc.vector.tensor_mul(mh[:], cont[:], sh[:])
            ha_new = tiny.tile([128, 1], f32, name=f"ha_{step}")
            nc.vector.tensor_add(ha_new[:], halt_acc[:], mh[:])
            pre = tiny.tile([128, 1], f32, name=f"pre_{step}")
            nc.vector.tensor_add(pre[:], ha_new[:], sh[:])
            endr = tiny.tile([128, 1], f32, name=f"endr_{step}")
            nc.vector.tensor_scalar(out=endr[:], in0=pre[:],
                                    scalar1=float(np.float32(1.0 - PONDER_EPS)),
                                    scalar2=None, op0=ALU.is_gt)
            ending = tiny.tile([128, 1], f32, name=f"end_{step}")
            nc.vector.tensor_mul(ending[:], cont[:], endr[:])
            cont_new = tiny.tile([128, 1], f32, name=f"cont_{step}")
            nc.vector.tensor_sub(cont_new[:], cont[:], ending[:])
            ts_new = tiny.tile([128, 1], f32, name=f"tsn_{step}")
            nc.vector.tensor_add(ts_new[:], tot_steps[:], cont_new[:])
            onemha = tiny.tile([128, 1], f32, name=f"omh_{step}")
            nc.vector.tensor_scalar(out=onemha[:], in0=ha_new[:], scalar1=-1.0,
                                    scalar2=1.0, op0=ALU.mult, op1=ALU.add)
            mr = tiny.tile([128, 1], f32, name=f"mr_{step}")
            nc.vector.tensor_mul(mr[:], ending[:], onemha[:])
            wsum = tiny.tile([128, 1], f32, name=f"w_{step}")
            nc.vector.tensor_add(wsum[:], mh[:], mr[:])
            wdt = tiny.tile([128, 1], f32, name=f"wdt_{step}")
            nc.vector.tensor_scalar(out=wdt[:], in0=wsum[:], scalar1=float(DT32),
                                    scalar2=None, op0=ALU.mult)
            tr_new = tiny.tile([128, 1], f32, name=f"trn_{step}")
            nc.vector.tensor_add(tr_new[:], tot_rem[:], mh[:])
            # tot_h += wdt * S   (h = dt*S)
            th_new = toth_bufs[1 - toth_cur]
            nc.vector.scalar_tensor_tensor(
                out=th_new[:], in0=s_now[:], scalar=wdt[:, 0:1],
                in1=toth_bufs[toth_cur][:], op0=ALU.mult, op1=ALU.add)
            toth_cur = 1 - toth_cur
            cont, halt_acc, tot_steps, tot_rem = cont_new, ha_new, ts_new, tr_new

        # ---------- final remainder term ----------
        onemha_f = tiny.tile([128, 1], f32, name="omh_f")
        nc.vector.tensor_scalar(out=onemha_f[:], in0=halt_acc[:], scalar1=-1.0,
                                scalar2=1.0, op0=ALU.mult, op1=ALU.add)
        wf = tiny.tile([128, 1], f32, name="wf")
        nc.vector.tensor_mul(wf[:], cont[:], onemha_f[:])
        wfdt = tiny.tile([128, 1], f32, name="wfdt")
        nc.vector.tensor_scalar(out=wfdt[:], in0=wf[:], scalar1=float(DT32),
                                scalar2=None, op0=ALU.mult)
        th_fin = toth_bufs[1 - toth_cur]
        nc.vector.scalar_tensor_tensor(
            out=th_fin[:], in0=s_bufs[s_cur][:], scalar=wfdt[:, 0:1],
            in1=toth_bufs[toth_cur][:], op0=ALU.mult, op1=ALU.add)

        if fold_a:
            th_conv = cp.tile([128, S], f32, name="th_conv")
            nc.vector.tensor_mul(th_conv[:], th_fin[:], ab_sb[:])
            th_fin = th_conv

        aux_sb = cp.tile([128, 4], f32, name="aux_sb")
        nc.vector.tensor_copy(aux_sb[:, 0:1], tot_rem[:])
        nc.vector.tensor_copy(aux_sb[:, 1:2], tot_steps[:])
        nc.vector.tensor_copy(aux_sb[:, 2:3], cont[:])
        nc.vector.tensor_copy(aux_sb[:, 3:4], halt_acc[:])

        nc.sync.dma_start(o_toth[:], th_fin[:])
        nc.sync.dma_start(o_aux[:], aux_sb[:])

    nc.compile()
    return nc


_BUILD_CACHE = {}


def _get_program(n_steps, w_terms, first_zero, tau_const, fold_a):
    key = (n_steps, w_terms, first_zero, tau_const, fold_a)
    if key not in _BUILD_CACHE:
        _BUILD_CACHE[key] = _build(n_steps, w_terms, first_zero, tau_const,
                                   fold_a=fold_a)
    return _BUILD_CACHE[key]


def _prep_inputs(inputs, hidden0, Wx, Wh, b, A, tau, w_halt, b_halt,
                 w_terms, first_zero, tau_const, fold_a):
    ws = (DT32 * Wh.astype(np.float64)).astype(np.float32)
    if fold_a:
        # T-space: fold A into weight rows / halting weights; state T = S/A
        ws = (A.astype(np.float64).reshape(-1, 1) * ws).astype(np.float32)
    whi = ws.astype(np.float16)
    wlo = (ws - whi.astype(np.float32)).astype(np.float16)
    wxhi = Wx.astype(np.float16)
    wxlo = (Wx - wxhi.astype(np.float32)).astype(np.float16)

    cpp = np.float32(1.0 + DT64 / tau.astype(np.float64))
    abv = A.astype(np.float32)          # S' = (S + A*f)*rho  (state scaled by 1/dt)
    whs64 = DT32 * w_halt.astype(np.float64).reshape(-1)
    if fold_a:
        whs64 = whs64 * A.astype(np.float64)
    whs = whs64.astype(np.float32).reshape(1, S)

    common = {
        "whi": whi, "wxhi": wxhi, "wxlo": wxlo,
        "ident": np.eye(128, dtype=np.float16),
        "identf": np.eye(128, dtype=np.float32),
        "bvec": b.reshape(1, S).astype(np.float32),
        "ones": np.ones((1, 128), np.float32),
        "ab": np.broadcast_to(abv.reshape(1, S), (128, S)).copy(),
        "whs": np.broadcast_to(whs, (128, S)).copy(),
        "bhalt": np.full((128, 1), np.float32(b_halt.reshape(-1)[0]), np.float32),
        "consts": np.broadcast_to(
            np.array([[cpp.flat[0] if tau_const else 0.0,
                       np.float32(1.0 - PONDER_EPS), DT32, 0.0]], np.float32),
            (128, 4)).copy(),
    }
    if w_terms >= 2:
        common["wlo"] = wlo
    if not tau_const:
        common["cb"] = np.broadcast_to(cpp.reshape(1, S), (128, S)).copy()

    in_maps = []
    for c in range(N_CORES):
        rows = slice(BL * c, BL * (c + 1))
        xs = inputs[rows, :]
        xT = np.ascontiguousarray(xs.T)
        xhiT = xT.astype(np.float16)
        xloT = (xT - xhiT.astype(np.float32)).astype(np.float16)
        m = dict(common)
        m["xhiT"] = xhiT
        m["xloT"] = xloT
        if not first_zero:
            s0 = (hidden0[rows, :].astype(np.float64) / DT64)
            if fold_a:
                s0 = s0 / A.astype(np.float64).reshape(1, -1)
            s0 = s0.astype(np.float32)
            s0hiT = np.ascontiguousarray(s0.T).astype(np.float16)
            m["s0"] = s0
            m["s0hiT"] = s0hiT
            if w_terms >= 3:
                m["s0loT"] = (np.ascontiguousarray(s0.T)
                              - s0hiT.astype(np.float32)).astype(np.float16)
        in_maps.append(m)
    return in_maps


def _run(n_steps, w_terms, first_zero, tau_const, in_maps, trace=False, fold_a=False):
    nc = _get_program(n_steps, w_terms, first_zero, tau_const, fold_a)
    res = run_bass_kernel_spmd(nc, in_maps, list(range(N_CORES)), trace=trace,
                               trace_cores=[0] if trace else None)
    return res


def kernel(inputs, hidden0, Wx, Wh, b, A, tau, w_halt, b_halt,
           _trace=False, _n_steps=N_STEPS_FAST, _w_terms=W_TERMS):
    inputs = np.asarray(inputs, np.float32)
    hidden0 = np.asarray(hidden0, np.float32)
    Wx = np.asarray(Wx, np.float32)
    Wh = np.asarray(Wh, np.float32)
    b = np.asarray(b, np.float32)
    A = np.asarray(A, np.float32)
    tau = np.asarray(tau, np.float32)
    w_halt = np.asarray(w_halt, np.float32)
    b_halt = np.asarray(b_halt, np.float32)

    first_zero = not hidden0.any()
    tau_const = bool(np.all(tau == tau.flat[0]))
    fold_a = bool(np.abs(A).min() >= 2e-4)   # |T|=|S/A| must stay well inside fp16

    in_maps = _prep_inputs(inputs, hidden0, Wx, Wh, b, A, tau, w_halt, b_halt,
                           _w_terms, first_zero, tau_const, fold_a)

    res = _run(_n_steps, _w_terms, first_zero, tau_const, in_maps, trace=_trace,
               fold_a=fold_a)
    results = res.results

    toth = np.concatenate([results[c]["tot_h"] for c in range(N_CORES)], axis=0)
    aux = np.concatenate([results[c]["aux"] for c in range(N_CORES)], axis=0)
    tot_rem, tot_steps, cont = aux[:, 0:1], aux[:, 1:2], aux[:, 2:3]

    if cont.any() and _n_steps < TIME_LIMIT - 1:
        # some rows did not halt inside the truncated scan: rerun full length
        res = _run(TIME_LIMIT - 1, _w_terms, first_zero, tau_const, in_maps,
                   trace=_trace, fold_a=fold_a)
        results = res.results
        toth = np.concatenate([results[c]["tot_h"] for c in range(N_CORES)], axis=0)
        aux = np.concatenate([results[c]["aux"] for c in range(N_CORES)], axis=0)
        tot_rem, tot_steps, cont = aux[:, 0:1], aux[:, 1:2], aux[:, 2:3]

    ponder_cost = np.float32(-TIME_PENALTY * tot_rem.astype(np.float64).mean())
    out_steps = (tot_steps + 1.0).astype(np.float32)
    kernel._last_exec_time_ns = getattr(res, "exec_time_ns", None)
    kernel._last_res = res
    return toth.astype(np.float32), ponder_cost, out_steps


# revision 27
# speedup vs baseline: 1.0686x; 1.0092x over previous
"""AdaptiveLTCCell Trainium2 kernel (8 NeuronCores, batch-parallel).

Reference math per scan iteration (3 semi-implicit Euler unfolds + ACT halting):
    f  = sigmoid(x@Wx + h@Wh + b)
    h' = (h + dt*f*A) / (1 + dt*(1/tau + f))
Device formulation uses the doubly-scaled state T = h/(dt*A) ("fold_a", used
whenever min|A| keeps T inside fp16 range) with W2 = diag(A)*dt*Wh:
    z   = fp16(T)@fp16(W2) + XP     (XP = x@Wx + b, computed once on device)
    f   = sigmoid(z)                 (ScalarE, from PSUM)
    rho = 1/(dt*f + 1 + dt/tau)     (reciprocal_approx_fast, 51 ULP)
    T'  = (T + f) * rho             (exact algebra: the A-multiply lives in W2)
    h'  = dt*A*T'  (dt folded into halting/output coefficients; A applied to
                    the accumulated tot_h once at the end)
Per unfold: fp16 matmuls accumulate in fp32 PSUM, PE transposes T' (fp32) and
ScalarE casts to fp16 for the next unfold's stationary operand; the elementwise
chain runs at 256-wide granularity so the five engines pipeline around the
serial recurrence. The scan is truncated to N_STEPS iterations; iterations past
the point where every batch row has halted contribute exactly zero to all
outputs. The kernel emits the final `cont` mask and falls back to a full-length
build if any row is still continuing (never happens for inputs where all rows
halt within N_STEPS). Validated: tot_steps matches the fp32 reference exactly;
tot_h max-abs error ~4e-4 on a 1.45-scale output.
"""
import numpy as np
from contextlib import ExitStack

import concourse.bass as bass
import concourse.tile as tile
from concourse import bacc, mybir
from concourse.bass_utils import run_bass_kernel_spmd

f32 = mybir.dt.float32
f16 = mybir.dt.float16
AF = mybir.ActivationFunctionType
ALU = mybir.AluOpType
AX = mybir.AxisListType

B, D_IN, S = 1024, 1024, 1024
TIME_LIMIT = 16
ODE_UNFOLDS = 3
PONDER_EPS = 0.01
TIME_PENALTY = 0.01
N_CORES = 8
BL = B // N_CORES          # local batch rows per core
N_STEPS_FAST = 5           # all rows halt within this for the target input (cont-checked)
W_TERMS = 1                # 1: z=hi@W16; 2: z=hi@Whi+hi@Wlo; 3: +lo@Whi

DT64 = 1.0 / ODE_UNFOLDS
DT32 = np.float32(DT64)


def _build(n_steps: int, w_terms: int, first_zero: bool, tau_const: bool,
           fold_a: bool = False):
    nc = bacc.Bacc("TRN2", target_bir_lowering=False, debug=False,
                   num_devices=N_CORES)

    dram = {}
    def din(name, shape, dt=f32):
        dram[name] = nc.dram_tensor(name, list(shape), dt, kind="ExternalInput").ap()
        return dram[name]

    # fp16 weight splits (host-prepared), k-tile t occupies rows 128t:128(t+1)
    d_whi = din("whi", [S, S], f16)
    if w_terms >= 2:
        d_wlo = din("wlo", [S, S], f16)
    d_wxhi = din("wxhi", [D_IN, S], f16)
    d_wxlo = din("wxlo", [D_IN, S], f16)
    d_xhiT = din("xhiT", [D_IN, BL], f16)
    d_xloT = din("xloT", [D_IN, BL], f16)
    d_ident = din("ident", [128, 128], f16)      # fp16 identity (for XP replay MMs)
    d_identf = din("identf", [128, 128], f32)    # fp32 identity (for transposes)
    d_bvec = din("bvec", [1, S])                 # bias b as a row
    d_ones = din("ones", [1, 128])
    d_ab = din("ab", [128, S])                   # broadcast dt*A... actually A (state scaled)
    d_whs = din("whs", [128, S])                 # broadcast dt*w_halt
    d_bhalt = din("bhalt", [128, 1])
    d_consts = din("consts", [128, 4])           # [c'', budget, dt, 0]
    if not tau_const:
        d_cb = din("cb", [128, S])               # broadcast c'' (general tau)
    if not first_zero:
        d_s0 = din("s0", [BL, S])                # initial S = hidden0/dt
        d_s0hiT = din("s0hiT", [S, BL], f16)
        if w_terms >= 3:
            d_s0loT = din("s0loT", [S, BL], f16)

    o_toth = nc.dram_tensor("tot_h", [BL, S], f32, kind="ExternalOutput").ap()
    o_aux = nc.dram_tensor("aux", [BL, 4], f32, kind="ExternalOutput").ap()

    with tile.TileContext(nc) as tc, ExitStack() as ctx:
        cp = ctx.enter_context(tc.tile_pool(name="cp", bufs=1))
        sp = ctx.enter_context(tc.tile_pool(name="sp", bufs=3))
        tiny = ctx.enter_context(tc.tile_pool(name="tiny", bufs=2))
        pz = ctx.enter_context(tc.tile_pool(name="pz", bufs=4, space="PSUM"))
        pt = ctx.enter_context(tc.tile_pool(name="pt", bufs=4, space="PSUM"))

        # ---------------- constant loads ----------------
        def load_tiled(dst, dsrc, free, dt_):
            half = 4 * free
            src3 = dsrc.rearrange("(t p) f -> p t f", p=128)
            nc.sync.dma_start(dst[:, 0:half].rearrange("p (t f) -> p t f", t=4),
                              src3[:, 0:4, :])
            nc.sync.dma_start(dst[:, half:2 * half].rearrange("p (t f) -> p t f", t=4),
                              src3[:, 4:8, :])
        whi_sb = cp.tile([128, 8 * S], f16)
        load_tiled(whi_sb[:], d_whi, S, f16)
        if w_terms >= 2:
            wlo_sb = cp.tile([128, 8 * S], f16)
            load_tiled(wlo_sb[:], d_wlo, S, f16)
        wxhi_sb = cp.tile([128, 8 * S], f16)
        load_tiled(wxhi_sb[:], d_wxhi, S, f16)
        xhiT_sb = cp.tile([128, 8 * BL], f16)
        load_tiled(xhiT_sb[:], d_xhiT, BL, f16)
        id16_sb = cp.tile([128, 128], f16)
        nc.sync.dma_start(id16_sb[:], d_ident[:])
        id32_sb = cp.tile([128, 128], f32)
        nc.sync.dma_start(id32_sb[:], d_identf[:])
        bvec_sb = cp.tile([1, S], f32)
        nc.sync.dma_start(bvec_sb[:], d_bvec[:])
        ones_sb = cp.tile([1, 128], f32)
        nc.sync.dma_start(ones_sb[:], d_ones[:])
        ab_sb = cp.tile([128, S], f32)
        nc.sync.dma_start(ab_sb[:], d_ab[:])
        whs_sb = cp.tile([128, S], f32)
        nc.sync.dma_start(whs_sb[:], d_whs[:])
        bhalt_sb = cp.tile([128, 1], f32)
        nc.sync.dma_start(bhalt_sb[:], d_bhalt[:])
        consts_sb = cp.tile([128, 4], f32)
        nc.sync.dma_start(consts_sb[:], d_consts[:])
        if not tau_const:
            cb_sb = cp.tile([128, S], f32)
            nc.sync.dma_start(cb_sb[:], d_cb[:])

        # ---------------- XP = x@Wx + b (once, fp16 3-term) ----------------
        xp_hi = cp.tile([128, S], f16)          # fp16 split of XP for replay
        xp_lo = cp.tile([128, S], f16)
        xp_f32 = cp.tile([128, S], f32)
        for bk in range(2):
            xpp = pz.tile([128, 512], f32, tag="z", name=f"xp_ps_{bk}")
            for k in range(8):
                nc.tensor.matmul(
                    xpp[:],
                    xhiT_sb[:, BL * k:BL * (k + 1)],
                    wxhi_sb[:, S * k + 512 * bk:S * k + 512 * bk + 512],
                    start=(k == 0), stop=(k == 7))
            # add bias b via K=1 outer product (rank-1), separate group
            nc.tensor.matmul(xpp[:], ones_sb[0:1, 0:128].bitcast(f32),
                             bvec_sb[0:1, 512 * bk:512 * bk + 512],
                             start=False, stop=True, skip_group_check=True)
            sl = slice(512 * bk, 512 * (bk + 1))
            nc.vector.tensor_copy(xp_f32[:, sl], xpp[:])
            nc.vector.tensor_copy(xp_hi[:, sl], xpp[:])
        nc.vector.tensor_sub(xp_lo[:], xp_f32[:], xp_hi[:])

        # ---------------- state init ----------------
        s_bufs = [cp.tile([128, S], f32, name="s_a"), cp.tile([128, S], f32, name="s_b")]
        sht_bufs = [cp.tile([128, S], f16, name="sht_a"), cp.tile([128, S], f16, name="sht_b")]
        if w_terms >= 3:
            slt_bufs = [cp.tile([128, S], f16, name="slt_a"), cp.tile([128, S], f16, name="slt_b")]
        toth_bufs = [cp.tile([128, S], f32, name="toth_a"), cp.tile([128, S], f32, name="toth_b")]
        if first_zero:
            nc.vector.memset(s_bufs[0][:], 0.0)
        else:
            nc.sync.dma_start(s_bufs[0][:], d_s0[:])
            load_tiled(sht_bufs[0][:], d_s0hiT, BL, f16)
            if w_terms >= 3:
                load_tiled(slt_bufs[0][:], d_s0loT, BL, f16)
        nc.vector.memset(toth_bufs[0][:], 0.0)

        cont = cp.tile([128, 1], f32, name="cont0")
        nc.vector.memset(cont[:], 1.0)
        halt_acc = cp.tile([128, 1], f32, name="ha0")
        nc.vector.memset(halt_acc[:], 0.0)
        tot_steps = cp.tile([128, 1], f32, name="ts0")
        nc.vector.memset(tot_steps[:], 0.0)
        tot_rem = cp.tile([128, 1], f32, name="tr0")
        nc.vector.memset(tot_rem[:], 0.0)

        cpp = consts_sb  # [c'', budget, dt, 0]
        s_cur = 0
        toth_cur = 0

        for step in range(n_steps):
            for u in range(ODE_UNFOLDS):
                uid = step * ODE_UNFOLDS + u
                skip_smm = first_zero and uid == 0
                f_sb = sp.tile([128, S], f32, name=f"f_{uid}", tag="f")
                den_sb = sp.tile([128, S], f32, name=f"den_{uid}", tag="den")
                rho_sb = sp.tile([128, S], f32, name=f"rho_{uid}", tag="rho")
                fa_sb = sp.tile([128, S], f32, name=f"fa_{uid}", tag="fa")
                num_sb = sp.tile([128, S], f32, name=f"num_{uid}", tag="num")
                s_new = s_bufs[1 - s_cur]
                sht_old = sht_bufs[s_cur]
                sht_new = sht_bufs[1 - s_cur]
                if w_terms >= 3:
                    slt_old = slt_bufs[s_cur]
                    slt_new = slt_bufs[1 - s_cur]
                s_old = s_bufs[s_cur]

                for bk in range(2):
                    zp = pz.tile([128, 512], f32, tag="z", name=f"z_{uid}_{bk}")
                    # XP replay (identity MMs, fp16 hi+lo)
                    nc.tensor.matmul(zp[:], id16_sb[:], xp_hi[:, 512 * bk:512 * bk + 512],
                                     start=True, stop=False)
                    last = skip_smm
                    nc.tensor.matmul(zp[:], id16_sb[:], xp_lo[:, 512 * bk:512 * bk + 512],
                                     start=False, stop=last)
                    if not skip_smm:
                        nmm = 0
                        tot_mm = 8 * w_terms
                        for k in range(8):
                            ksl = slice(128 * k, 128 * (k + 1))
                            wsl = slice(S * k + 512 * bk, S * k + 512 * bk + 512)
                            nmm += 1
                            nc.tensor.matmul(zp[:], sht_old[:, ksl], whi_sb[:, wsl],
                                             start=False, stop=(nmm == tot_mm))
                            if w_terms >= 2:
                                nmm += 1
                                nc.tensor.matmul(zp[:], sht_old[:, ksl], wlo_sb[:, wsl],
                                                 start=False, stop=(nmm == tot_mm))
                            if w_terms >= 3:
                                nmm += 1
                                nc.tensor.matmul(zp[:], slt_old[:, ksl], whi_sb[:, wsl],
                                                 start=False, stop=(nmm == tot_mm))
                    last_u = (step == n_steps - 1 and u == ODE_UNFOLDS - 1)
                    sl = slice(512 * bk, 512 * (bk + 1))
                    stp = None
                    if not last_u:
                        stp = pt.tile([128, 512], f32, tag="st", name=f"st_{uid}_{bk}")
                    for q in range(2):
                        qsl = slice(512 * bk + 256 * q, 512 * bk + 256 * (q + 1))
                        zq = slice(256 * q, 256 * (q + 1))
                        # f = sigmoid(z)
                        nc.scalar.activation(f_sb[:, qsl], zp[:, zq], AF.Sigmoid)
                        # den = dt*f + (1 + dt/tau)
                        if tau_const:
                            nc.scalar.activation(den_sb[:, qsl], f_sb[:, qsl], AF.Identity,
                                                 scale=cpp[:, 2:3], bias=cpp[:, 0:1])
                        else:
                            nc.vector.scalar_tensor_tensor(
                                out=den_sb[:, qsl], in0=f_sb[:, qsl], scalar=float(DT32),
                                in1=cb_sb[:, qsl], op0=ALU.mult, op1=ALU.add)
                        # rho ~= 1/den (51-ULP fast reciprocal)
                        nc.vector.reciprocal_approx_fast(out=rho_sb[:, qsl], in_=den_sb[:, qsl])
                        if fold_a:
                            # T-space: T' = (T + f) * rho  (A folded into weights)
                            if skip_smm and first_zero:
                                nc.vector.tensor_mul(s_new[:, qsl], f_sb[:, qsl], rho_sb[:, qsl])
                            else:
                                nc.vector.tensor_add(num_sb[:, qsl], f_sb[:, qsl], s_old[:, qsl])
                                nc.vector.tensor_mul(s_new[:, qsl], num_sb[:, qsl], rho_sb[:, qsl])
                        else:
                            # fa = f * A ; num = S + fa ; S' = num * rho
                            nc.vector.tensor_mul(fa_sb[:, qsl], f_sb[:, qsl], ab_sb[:, qsl])
                            if skip_smm and first_zero:
                                nc.vector.tensor_mul(s_new[:, qsl], fa_sb[:, qsl], rho_sb[:, qsl])
                            else:
                                nc.vector.tensor_add(num_sb[:, qsl], fa_sb[:, qsl], s_old[:, qsl])
                                nc.vector.tensor_mul(s_new[:, qsl], num_sb[:, qsl], rho_sb[:, qsl])
                        # transpose S' tiles (fp32) and cast to fp16 for next unfold
                        if not last_u:
                            for t2 in range(2):
                                t4 = 2 * q + t2
                                t = 4 * bk + t4
                                nc.tensor.matmul(
                                    stp[:, 128 * t4:128 * (t4 + 1)],
                                    s_new[:, 128 * t:128 * (t + 1)], id32_sb[:],
                                    is_transpose=True, start=True, stop=True)
                            nc.scalar.activation(
                                sht_new[:, qsl], stp[:, 256 * q:256 * (q + 1)], AF.Copy)
                            if w_terms >= 3:
                                nc.vector.tensor_sub(slt_new[:, qsl],
                                                     stp[:, 256 * q:256 * (q + 1)],
                                                     sht_new[:, qsl])
                s_cur = 1 - s_cur

            # ---------- halting (end of step) ----------
            s_now = s_bufs[s_cur]
            hv = tiny.tile([128, 1], f32, name=f"hv_{step}")
            ttr_junk = sp.tile([128, S], f32, name=f"ttrj_{step}", tag="ttrj")
            nc.vector.affine_mul_reduce(out=ttr_junk[:], accum_out=hv[:],
                                        in0=s_now[:], in1=whs_sb[:],
                                        scale=1.0, bias=0.0)
            sh = tiny.tile([128, 1], f32, name=f"sh_{step}")
            nc.scalar.activation(sh[:], hv[:], AF.Sigmoid, bias=bhalt_sb[:, 0:1])
            mh = tiny.tile([128, 1], f32, name=f"mh_{step}")
            nc.vector.tensor_mul(mh[:], cont[:], sh[:])
            ha_new = tiny.tile([128, 1], f32, name=f"ha_{step}")
            nc.vector.tensor_add(ha_new[:], halt_acc[:], mh[:])
            pre = tiny.tile([128, 1], f32, name=f"pre_{step}")
            nc.vector.tensor_add(pre[:], ha_new[:], sh[:])
            endr = tiny.tile([128, 1], f32, name=f"endr_{step}")
            nc.vector.tensor_scalar(out=endr[:], in0=pre[:],
                                    scalar1=float(np.float32(1.0 - PONDER_EPS)),
                                    scalar2=None, op0=ALU.is_gt)
            ending = tiny.tile([128, 1], f32, name=f"end_{step}")
            nc.vector.tensor_mul(ending[:], cont[:], endr[:])
            cont_new = tiny.tile([128, 1], f32, name=f"cont_{step}")
            nc.vector.tensor_sub(cont_new[:], cont[:], ending[:])
            ts_new = tiny.tile([128, 1], f32, name=f"tsn_{step}")
            nc.vector.tensor_add(ts_new[:], tot_steps[:], cont_new[:])
            onemha = tiny.tile([128, 1], f32, name=f"omh_{step}")
            nc.vector.tensor_scalar(out=onemha[:], in0=ha_new[:], scalar1=-1.0,
                                    scalar2=1.0, op0=ALU.mult, op1=ALU.add)
            mr = tiny.tile([128, 1], f32, name=f"mr_{step}")
            nc.vector.tensor_mul(mr[:], ending[:], onemha[:])
            wsum = tiny.tile([128, 1], f32, name=f"w_{step}")
            nc.vector.tensor_add(wsum[:], mh[:], mr[:])
            wdt = tiny.tile([128, 1], f32, name=f"wdt_{step}")
            nc.vector.tensor_scalar(out=wdt[:], in0=wsum[:], scalar1=float(DT32),
                                    scalar2=None, op0=ALU.mult)
            tr_new = tiny.tile([128, 1], f32, name=f"trn_{step}")
            nc.vector.tensor_add(tr_new[:], tot_rem[:], mh[:])
            # tot_h += wdt * S   (h = dt*S)
            th_new = toth_bufs[1 - toth_cur]
            nc.vector.scalar_tensor_tensor(
                out=th_new[:], in0=s_now[:], scalar=wdt[:, 0:1],
                in1=toth_bufs[toth_cur][:], op0=ALU.mult, op1=ALU.add)
            toth_cur = 1 - toth_cur
            cont, halt_acc, tot_steps, tot_rem = cont_new, ha_new, ts_new, tr_new

        # ---------- final remainder term ----------
        onemha_f = tiny.tile([128, 1], f32, name="omh_f")
        nc.vector.tensor_scalar(out=onemha_f[:], in0=halt_acc[:], scalar1=-1.0,
                                scalar2=1.0, op0=ALU.mult, op1=ALU.add)
        wf = tiny.tile([128, 1], f32, name="wf")
        nc.vector.tensor_mul(wf[:], cont[:], onemha_f[:])
        wfdt = tiny.tile([128, 1], f32, name="wfdt")
        nc.vector.tensor_scalar(out=wfdt[:], in0=wf[:], scalar1=float(DT32),
                                scalar2=None, op0=ALU.mult)
        th_fin = toth_bufs[1 - toth_cur]
        nc.vector.scalar_tensor_tensor(
            out=th_fin[:], in0=s_bufs[s_cur][:], scalar=wfdt[:, 0:1],
            in1=toth_bufs[toth_cur][:], op0=ALU.mult, op1=ALU.add)

        if fold_a:
            th_conv = cp.tile([128, S], f32, name="th_conv")
            nc.vector.tensor_mul(th_conv[:], th_fin[:], ab_sb[:])
            th_fin = th_conv

        aux_sb = cp.tile([128, 4], f32, name="aux_sb")
        nc.vector.tensor_copy(aux_sb[:, 0:1], tot_rem[:])
        nc.vector.tensor_copy(aux_sb[:, 1:2], tot_steps[:])
        nc.vector.tensor_copy(aux_sb[:, 2:3], cont[:])
        nc.vector.tensor_copy(aux_sb[:, 3:4], halt_acc[:])

        nc.sync.dma_start(o_toth[:], th_fin[:])
        nc.sync.dma_start(o_aux[:], aux_sb[:])

    nc.compile()
    return nc


_BUILD_CACHE = {}


def _get_program(n_steps, w_terms, first_zero, tau_const, fold_a):
    key = (n_steps, w_terms, first_zero, tau_const, fold_a)
    if key not in _BUILD_CACHE:
        _BUILD_CACHE[key] = _build(n_steps, w_terms, first_zero, tau_const,
                                   fold_a=fold_a)
    return _BUILD_CACHE[key]


def _prep_inputs(inputs, hidden0, Wx, Wh, b, A, tau, w_halt, b_halt,
                 w_terms, first_zero, tau_const, fold_a):
    ws = (DT32 * Wh.astype(np.float64)).astype(np.float32)
    if fold_a:
        # T-space: fold A into weight rows / halting weights; state T = S/A
        ws = (A.astype(np.float64).reshape(-1, 1) * ws).astype(np.float32)
    whi = ws.astype(np.float16)
    wlo = (ws - whi.astype(np.float32)).astype(np.float16)
    wxhi = Wx.astype(np.float16)
    wxlo = (Wx - wxhi.astype(np.float32)).astype(np.float16)

    cpp = np.float32(1.0 + DT64 / tau.astype(np.float64))
    abv = A.astype(np.float32)          # S' = (S + A*f)*rho  (state scaled by 1/dt)
    whs64 = DT32 * w_halt.astype(np.float64).reshape(-1)
    if fold_a:
        whs64 = whs64 * A.astype(np.float64)
    whs = whs64.astype(np.float32).reshape(1, S)

    common = {
        "whi": whi, "wxhi": wxhi, "wxlo": wxlo,
        "ident": np.eye(128, dtype=np.float16),
        "identf": np.eye(128, dtype=np.float32),
        "bvec": b.reshape(1, S).astype(np.float32),
        "ones": np.ones((1, 128), np.float32),
        "ab": np.broadcast_to(abv.reshape(1, S), (128, S)).copy(),
        "whs": np.broadcast_to(whs, (128, S)).copy(),
        "bhalt": np.full((128, 1), np.float32(b_halt.reshape(-1)[0]), np.float32),
        "consts": np.broadcast_to(
            np.array([[cpp.flat[0] if tau_const else 0.0,
                       np.float32(1.0 - PONDER_EPS), DT32, 0.0]], np.float32),
            (128, 4)).copy(),
    }
    if w_terms >= 2:
        common["wlo"] = wlo
    if not tau_const:
        common["cb"] = np.broadcast_to(cpp.reshape(1, S), (128, S)).copy()

    in_maps = []
    for c in range(N_CORES):
        rows = slice(BL * c, BL * (c + 1))
        xs = inputs[rows, :]
        xT = np.ascontiguousarray(xs.T)
        xhiT = xT.astype(np.float16)
        xloT = (xT - xhiT.astype(np.float32)).astype(np.float16)
        m = dict(common)
        m["xhiT"] = xhiT
        m["xloT"] = xloT
        if not first_zero:
            s0 = (hidden0[rows, :].astype(np.float64) / DT64)
            if fold_a:
                s0 = s0 / A.astype(np.float64).reshape(1, -1)
            s0 = s0.astype(np.float32)
            s0hiT = np.ascontiguousarray(s0.T).astype(np.float16)
            m["s0"] = s0
            m["s0hiT"] = s0hiT
            if w_terms >= 3:
                m["s0loT"] = (np.ascontiguousarray(s0.T)
                              - s0hiT.astype(np.float32)).astype(np.float16)
        in_maps.append(m)
    return in_maps


def _run(n_steps, w_terms, first_zero, tau_const, in_maps, trace=False, fold_a=False):
    nc = _get_program(n_steps, w_terms, first_zero, tau_const, fold_a)
    res = run_bass_kernel_spmd(nc, in_maps, list(range(N_CORES)), trace=trace,
                               trace_cores=[0] if trace else None)
    return res


def kernel(inputs, hidden0, Wx, Wh, b, A, tau, w_halt, b_halt,
           _trace=False, _n_steps=N_STEPS_FAST, _w_terms=W_TERMS):
    inputs = np.asarray(inputs, np.float32)
    hidden0 = np.asarray(hidden0, np.float32)
    Wx = np.asarray(Wx, np.float32)
    Wh = np.asarray(Wh, np.float32)
    b = np.asarray(b, np.float32)
    A = np.asarray(A, np.float32)
    tau = np.asarray(tau, np.float32)
    w_halt = np.asarray(w_halt, np.float32)
    b_halt = np.asarray(b_halt, np.float32)

    first_zero = not hidden0.any()
    tau_const = bool(np.all(tau == tau.flat[0]))
    fold_a = bool(np.abs(A).min() >= 2e-4)   # |T|=|S/A| must stay well inside fp16

    in_maps = _prep_inputs(inputs, hidden0, Wx, Wh, b, A, tau, w_halt, b_halt,
                           _w_terms, first_zero, tau_const, fold_a)

    res = _run(_n_steps, _w_terms, first_zero, tau_const, in_maps, trace=_trace,
               fold_a=fold_a)
    results = res.results

    toth = np.concatenate([results[c]["tot_h"] for c in range(N_CORES)], axis=0)
    aux = np.concatenate([results[c]["aux"] for c in range(N_CORES)], axis=0)
    tot_rem, tot_steps, cont = aux[:, 0:1], aux[:, 1:2], aux[:, 2:3]

    if cont.any() and _n_steps < TIME_LIMIT - 1:
        # some rows did not halt inside the truncated scan: rerun full length
        res = _run(TIME_LIMIT - 1, _w_terms, first_zero, tau_const, in_maps,
                   trace=_trace, fold_a=fold_a)
        results = res.results
        toth = np.concatenate([results[c]["tot_h"] for c in range(N_CORES)], axis=0)
        aux = np.concatenate([results[c]["aux"] for c in range(N_CORES)], axis=0)
        tot_rem, tot_steps, cont = aux[:, 0:1], aux[:, 1:2], aux[:, 2:3]

    ponder_cost = np.float32(-TIME_PENALTY * tot_rem.astype(np.float64).mean())
    out_steps = (tot_steps + 1.0).astype(np.float32)
    kernel._last_exec_time_ns = getattr(res, "exec_time_ns", None)
    kernel._last_res = res
    return toth.astype(np.float32), ponder_cost, out_steps
